# revision 31
# baseline (speedup 1.0000x reference)
"""Trainium2 Bass kernel for nn_Attention (dense transformer block):
y = Attention(RoPE(x@wqT), RoPE(x@wkT), x@wvT, causal) @ woT

Sharding: 8 cores = 2 batches x 4 head-groups (tensor-parallel heads,
data-parallel batch).  Each core handles one batch and 4 of the 16 heads
(512 of the 2048 channels): column-shard of wq/wk/wv, row-shard of wo.
Each core emits a full-shape [S, D] partial of y; the host sums the 4
partials per batch.

Fused chunk pipeline (per core, SPMD): for each 512-row seq chunk j,
  proj(j):  q/k (+RoPE on DVE) and v for chunk j from ONE pass over x
            tiles (v reuses the same SBUF x tiles as stationary
            operands — x is streamed from DRAM exactly once, in bf16)
  attn(j):  causal attention for all 4 heads over keys 0..j, transposed
            scores (sT[sk,sq] = kT.T @ qT, fp32r), exp on ACT -> bf16
            probs, causal mask + denominator tile-accumulation as DVE
            elementwise ops (one ones-matmul per head reduces the
            accumulator over partitions), software-pipelined with a
            3-tile lookahead so the PE never waits on exp
  wo(j):    row-parallel wo partial, nn-inner loop so each stationary
            out_h slice is loaded once for 4 matmuls (LDWEIGHTS
            amortization); psum borrowed from the idle score pool;
            interleaved with proj(j+1)'s v chains
Chunk j+1's x tiles prefetch during attn(j); all weights stay resident.
Everything is sized against HW-calibrated costs: matmul ~ N/2.4GHz +
~90ns dispatch + ~50ns on stationary rotation; per-instruction count,
not FLOPs, is the limiting budget beyond the 290us streaming floor.
"""

import os
import sys

import numpy as np

for _p in ("/opt/trn_rl_repo", "/root/.axon_site/_ro/trn_rl_repo"):
    if os.path.isdir(_p) and _p not in sys.path:
        sys.path.insert(0, _p)

import concourse.bass as bass
import concourse.tile as tile
from concourse import bacc
from concourse import mybir
from concourse import bass_utils

B, S, D, H = 2, 2048, 2048, 16
HD = 128                 # head dim
HPC = 4                  # heads per core
CPB = 4                  # cores per batch
N_CORES = 8
NK = D // 128            # 16 contraction chunks
NSQ = S // 512           # 4 sq chunks of 512
NSK = S // 128           # 16 sk tiles of 128
SCALE = float(1.0 / np.sqrt(np.float32(HD)))

F32 = mybir.dt.float32
F32R = mybir.dt.float32r
BF16 = mybir.dt.bfloat16

EXP = mybir.ActivationFunctionType.Exp
SWAP_MASK = [i ^ 1 for i in range(32)]

# diag-tile mask pattern m: columns < 128*m fully masked; keep score width
# >= 256 so the fp32r moving operand stays at full rate
DIAG_OFF = {0: 0, 1: 128, 2: 256, 3: 256}
LOOKAHEAD = 3


def _emit(tc):
    nc = tc.nc

    xT = nc.dram_tensor("xT", [D, S], BF16, kind="ExternalInput").ap()
    wqT = nc.dram_tensor("wqT", [D, HPC * HD], BF16, kind="ExternalInput").ap()
    wkT = nc.dram_tensor("wkT", [D, HPC * HD], BF16, kind="ExternalInput").ap()
    wvT = nc.dram_tensor("wvT", [D, HPC * HD], BF16, kind="ExternalInput").ap()
    woT = nc.dram_tensor("woT", [HPC * HD, D], BF16, kind="ExternalInput").ap()
    cosq = nc.dram_tensor("cosq", [HD, S], F32, kind="ExternalInput").ap()
    sinq = nc.dram_tensor("sinq", [HD, S], F32, kind="ExternalInput").ap()
    mask01 = nc.dram_tensor("mask01", [128, 4, 512], BF16, kind="ExternalInput").ap()
    onesd = nc.dram_tensor("onesd", [128, 128], F32R, kind="ExternalInput").ap()
    y = nc.dram_tensor("y", [S, D], BF16, kind="ExternalOutput").ap()

    ld = nc.sync        # all loads on the SP HWDGE queue
    st = nc.scalar      # y stores on the ACT HWDGE queue

    # ---- SBUF pools (all live for the whole kernel)
    consts = tc.alloc_tile_pool(name="consts", bufs=1)
    mask_sb = consts.tile([128, 4, 512], BF16, name="mask_sb")
    ones_sb = consts.tile([128, 128], F32R, name="ones_sb")

    wpool = tc.alloc_tile_pool(name="wpool", bufs=1)
    wq_sb = wpool.tile([128, NK, HPC * HD], BF16, name="wq_sb")
    wk_sb = wpool.tile([128, NK, HPC * HD], BF16, name="wk_sb")
    wv_sb = wpool.tile([128, NK, HPC * HD], BF16, name="wv_sb")
    wo_sb = wpool.tile([128, HPC, D], BF16, name="wo_sb")

    kpool = tc.alloc_tile_pool(name="kpool", bufs=1)
    kT = [[kpool.tile([128, 512], F32R, name=f"kT{h}_{j}") for j in range(NSQ)]
          for h in range(HPC)]
    vpool = tc.alloc_tile_pool(name="vpool", bufs=1)
    v_sb = [vpool.tile([128, HPC * HD], BF16, name=f"v{m}") for m in range(NSK)]

    qpool = tc.alloc_tile_pool(name="qpool", bufs=1)
    ropec = tc.alloc_tile_pool(name="ropec", bufs=2)
    xpool = tc.alloc_tile_pool(name="xpool", bufs=2)
    tpool = tc.alloc_tile_pool(name="tpool", bufs=2)
    ptpool = tc.alloc_tile_pool(name="ptpool", bufs=6)
    dpool = tc.alloc_tile_pool(name="dpool", bufs=2)
    opool = tc.alloc_tile_pool(name="opool", bufs=1)
    rpool = tc.alloc_tile_pool(name="rpool", bufs=2)
    ypool = tc.alloc_tile_pool(name="ypool", bufs=2)

    pg = tc.alloc_tile_pool(name="pg", bufs=2, space="PSUM")
    pss = tc.alloc_tile_pool(name="pss", bufs=LOOKAHEAD + 1, space="PSUM")
    pspv = tc.alloc_tile_pool(name="pspv", bufs=2, space="PSUM")

    # ---- prologue DMAs; xs/wq split in halves so the first chain starts
    # after ~1/4 of the lead-in bytes.  Loads split over both HWDGE queues.
    def load_x(j, nsplit=2):
        xt = xpool.tile([128, NK, 512], BF16, name=f"xs{j}", tag="xs")
        src = xT[:, 512 * j:512 * (j + 1)].rearrange("(kt p) c -> p kt c", p=128)
        step = NK // nsplit
        for q in range(nsplit):
            ld.dma_start(
                out=xt[:, step * q:step * (q + 1), :],
                in_=src[:, step * q:step * (q + 1), :],
            )
        return xt

    def load_rope(j):
        ct = ropec.tile([128, 512], F32, name=f"cos{j}", tag="cos")
        st.dma_start(out=ct, in_=cosq[:, 512 * j:512 * (j + 1)])
        sn = ropec.tile([128, 512], F32, name=f"sin{j}", tag="sin")
        st.dma_start(out=sn, in_=sinq[:, 512 * j:512 * (j + 1)])
        return ct, sn

    xs = load_x(0, nsplit=4)
    wq_src = wqT.rearrange("(kt p) c -> p kt c", p=128)
    for q in range(4):
        st.dma_start(
            out=wq_sb[:, 4 * q:4 * (q + 1), :], in_=wq_src[:, 4 * q:4 * (q + 1), :]
        )
    ld.dma_start(out=wk_sb, in_=wkT.rearrange("(kt p) c -> p kt c", p=128))
    ct0 = ropec.tile([128, 512], F32, name="cos0", tag="cos")
    ld.dma_start(out=ct0, in_=cosq[:, 0:512])
    sn0 = ropec.tile([128, 512], F32, name="sin0", tag="sin")
    ld.dma_start(out=sn0, in_=sinq[:, 0:512])
    cs0 = (ct0, sn0)
    ld.dma_start(out=wv_sb, in_=wvT.rearrange("(kt p) c -> p kt c", p=128))
    st.dma_start(out=mask_sb, in_=mask01)
    st.dma_start(out=ones_sb, in_=onesd)
    ld.dma_start(out=wo_sb, in_=woT.rearrange("(h p) d -> p h d", p=128))

    qT = [None] * HPC    # per-chunk q tiles, rewritten each chunk
    out_h = [None] * HPC

    def proj_chains(j, xs, cos_sb, sin_sb):
        """Closures: 8 q/k chains (+RoPE drain on DVE), 4 v chains (ACT
        drain).  The psum is freed by the 2nd DVE op (t2) for q/k, and by
        the single ACT copy for v."""
        chains = []

        def qk_chain(which, w_sb, h):
            def emit():
                acc = pg.tile([128, 512], F32, name=f"a{which}{j}_{h}", tag="pg")
                for k in range(NK):
                    nc.tensor.matmul(
                        acc, w_sb[:, k, 128 * h:128 * (h + 1)], xs[:, k, :],
                        start=(k == 0), stop=(k == NK - 1),
                    )
                if which == "q":
                    dst = qpool.tile([128, 512], F32R, name=f"qT{h}_{j}", tag=f"q{h}")
                    qT[h] = dst
                else:
                    dst = kT[h][j]
                shuf = tpool.tile([128, 512], F32, name=f"sh{which}{j}_{h}", tag="shuf")
                nc.vector.stream_shuffle(shuf, acc, SWAP_MASK)
                t2 = tpool.tile([128, 512], F32, name=f"t2{which}{j}_{h}", tag="t1")
                nc.vector.tensor_mul(t2, acc, cos_sb)
                t1 = tpool.tile([128, 512], F32, name=f"t1{which}{j}_{h}", tag="shuf")
                nc.vector.tensor_mul(t1, shuf, sin_sb)
                nc.vector.tensor_add(dst, t1, t2)
            return emit

        def v_chain(m):
            def emit():
                acc = pg.tile([128, HPC * HD], F32, name=f"av{j}_{m}", tag="pg")
                for k in range(NK):
                    nc.tensor.matmul(
                        acc, xs[:, k, 128 * m:128 * (m + 1)], wv_sb[:, k, :],
                        start=(k == 0), stop=(k == NK - 1),
                    )
                nc.scalar.copy(v_sb[4 * j + m], acc)
            return emit

        for which, w_sb in (("q", wq_sb), ("k", wk_sb)):
            for h in range(HPC):
                chains.append(qk_chain(which, w_sb, h))
        for m in range(4):
            chains.append(v_chain(m))
        return chains

    def emit_attn(j):
        """Causal attention for all heads over keys 0..4j+3, pipelined.
        Denominator = running sum of prob tiles on the DVE (per-head SBUF
        accumulator); causal mask = DVE multiply after the exp."""
        nsk = 4 * j + 4
        # flat tile list: per head, off-diagonal tiles first, then diagonal
        tiles = []
        for h in range(HPC):
            order = list(range(0, 4 * j)) + list(range(4 * j, nsk))
            for pos, i in enumerate(order):
                tiles.append((h, i, pos, pos == nsk - 1))
        n = len(tiles)
        st_of = {}   # tile idx -> (pt, off)
        pv_of = {}
        dacc_of = {}

        def emit_score(t):
            h, i, pos, last = tiles[t]
            off = DIAG_OFF[i - 4 * j] if i >= 4 * j else 0
            cs = slice(off, 512)
            s_ps = pss.tile([128, 512], F32, name=f"s{j}_{t}", tag="s")
            nc.tensor.matmul(
                s_ps[:, cs], kT[h][i // 4][:, 128 * (i % 4):128 * (i % 4 + 1)],
                qT[h][:, cs], start=True, stop=True,
            )
            pt = ptpool.tile([128, 512], BF16, name=f"p{j}_{t}", tag="pt")
            nc.scalar.activation(pt[:, cs], s_ps[:, cs], EXP, bias=0.0, scale=SCALE)
            if i >= 4 * j:
                meng = nc.vector if h % 2 == 0 else nc.gpsimd
                meng.tensor_mul(pt[:, cs], pt[:, cs], mask_sb[:, i - 4 * j, cs])
            # denominator running sum; heads 0,2 on DVE, heads 1,3 on the
            # otherwise-idle GpSimd (pos 0 tile is always full-width)
            deng = nc.vector if h % 2 == 0 else nc.gpsimd
            if pos == 0:
                dacc_of[h] = dpool.tile([128, 512], F32R, name=f"da{j}_{h}", tag="dacc")
                deng.tensor_copy(out=dacc_of[h], in_=pt)
            else:
                deng.tensor_add(dacc_of[h][:, cs], dacc_of[h][:, cs], pt[:, cs])
            st_of[t] = (pt, off)

        def emit_pv(t):
            h, i, pos, last = tiles[t]
            pt, off = st_of.pop(t)
            cs = slice(off, 512)
            if pos == 0:
                pv_of[h] = pspv.tile([128, 512], F32, name=f"pv{j}_{h}", tag="pv")
            nc.tensor.matmul(
                pv_of[h][:, cs], v_sb[i][:, 128 * h:128 * (h + 1)], pt[:, cs],
                start=(pos == 0), stop=last,
            )
            if last:
                den_ps = pss.tile([128, 512], F32, name=f"dn{j}_{h}", tag="s")
                nc.tensor.matmul(den_ps, ones_sb, dacc_of[h], start=True, stop=True)
                recip = rpool.tile([128, 512], F32, name=f"rc{j}_{h}", tag="recip")
                nc.vector.reciprocal(recip, den_ps)
                oh = opool.tile([128, 512], BF16, name=f"oh{j}_{h}", tag=f"o{h}")
                nc.vector.tensor_mul(oh, pv_of[h], recip)
                out_h[h] = oh

        for t in range(min(LOOKAHEAD, n)):
            emit_score(t)
        for t in range(n):
            if t + LOOKAHEAD < n:
                emit_score(t + LOOKAHEAD)
            emit_pv(t)

    def wo_groups(j):
        """Closures: 4 wo row-blocks.  Within a block the stationary
        out_h[h] slice is loaded once and streams all 4 col-slices (nn
        inner), accumulating into 4 psum banks borrowed from the (idle
        during wo) score pool."""
        groups = []

        def block(tt):
            def emit():
                ys = ypool.tile([128, D], BF16, name=f"ys{j}_{tt}", tag="ys")
                accs = [
                    pss.tile([128, 512], F32, name=f"wy{j}_{tt}_{nn}", tag="s")
                    for nn in range(4)
                ]
                for h in range(HPC):
                    for nn in range(4):
                        nc.tensor.matmul(
                            accs[nn],
                            out_h[h][:, 128 * tt:128 * (tt + 1)],
                            wo_sb[:, h, 512 * nn:512 * (nn + 1)],
                            start=(h == 0), stop=(h == HPC - 1),
                        )
                for nn in range(4):
                    dst = ys[:, 512 * nn:512 * (nn + 1)]
                    if nn % 2 == 0:
                        nc.vector.tensor_copy(out=dst, in_=accs[nn])
                    else:
                        nc.scalar.copy(dst, accs[nn])
                row = 512 * j + 128 * tt
                st.dma_start(out=y[row:row + 128, :], in_=ys)
            return emit

        for tt in range(4):
            groups.append(block(tt))
        return groups

    # Steady state per chunk j: attn(j) -> proj(j+1) q/k chains (their long
    # RoPE drains never sit between two short psum groups) -> v chains of
    # proj(j+1) interleaved 1:4 with wo(j) groups (v's ACT-copy drain frees
    # its psum fast enough for the interleave).
    for c in proj_chains(0, xs, *cs0):
        c()
    for j in range(NSQ):
        if j + 1 < NSQ:
            xs_next = load_x(j + 1)
            cs_next = load_rope(j + 1)
        emit_attn(j)
        wo = wo_groups(j)
        if j + 1 < NSQ:
            chains = proj_chains(j + 1, xs_next, *cs_next)
            for c in chains[:8]:      # q/k chains
                c()
            for m in range(4):        # v chains, wo blocks woven between
                chains[8 + m]()
                wo[m]()
        else:
            for g in wo:
                g()

    for p in (pspv, pss, pg, ypool, rpool, opool, dpool, ptpool, tpool,
              xpool, ropec, qpool, vpool, kpool, wpool, consts):
        p.release()


_PROGRAM = None


def build_program():
    global _PROGRAM
    if _PROGRAM is None:
        nc = bacc.Bacc("TRN2", target_bir_lowering=False, debug=False)
        with tile.TileContext(nc) as tc:
            _emit(tc)
        nc.compile()
        _PROGRAM = nc
    return _PROGRAM


def make_core_inputs(x, freqs_cos, freqs_sin, wq, wk, wv, wo):
    """Host-side sharding: returns list of 8 per-core input dicts."""
    import ml_dtypes

    bf16 = ml_dtypes.bfloat16
    x = np.asarray(x, dtype=np.float32)
    freqs_cos = np.asarray(freqs_cos, dtype=np.float32)
    freqs_sin = np.asarray(freqs_sin, dtype=np.float32)
    wq = np.asarray(wq, dtype=np.float32)
    wk = np.asarray(wk, dtype=np.float32)
    wv = np.asarray(wv, dtype=np.float32)
    wo = np.asarray(wo, dtype=np.float32)

    cosq = np.ascontiguousarray(np.repeat(freqs_cos.T, 2, axis=0))  # [128, S]
    sinq = np.ascontiguousarray(np.repeat(freqs_sin.T, 2, axis=0))
    sinq[0::2, :] *= -1.0  # even rows: -sin; odd rows: +sin

    skl = np.arange(128)[:, None]
    sql = np.arange(512)[None, :]
    mask01 = np.stack(
        [(128 * m + skl <= sql).astype(bf16) for m in range(4)], axis=1
    )  # [128, 4, 512]

    xTs = [np.ascontiguousarray(x[b].T).astype(bf16) for b in range(B)]
    in_maps = []
    for c in range(N_CORES):
        b, g = divmod(c, CPB)
        hsl = slice(512 * g, 512 * (g + 1))
        in_maps.append(
            {
                "xT": xTs[b],
                "wqT": np.ascontiguousarray(wq[hsl, :].T).astype(bf16),
                "wkT": np.ascontiguousarray(wk[hsl, :].T).astype(bf16),
                "wvT": np.ascontiguousarray(wv[hsl, :].T).astype(bf16),
                "woT": np.ascontiguousarray(wo[:, hsl].T).astype(bf16),
                "cosq": cosq,
                "sinq": sinq,
                "mask01": mask01,
                "onesd": np.ones((128, 128), dtype=np.float32),
            }
        )
    return in_maps


def run(inputs, trace=False, **spmd_kwargs):
    """Run the SPMD kernel on 8 cores.  Returns (y_full, BassKernelResults)."""
    nc = build_program()
    in_maps = make_core_inputs(
        inputs["x"], inputs["freqs_cos"], inputs["freqs_sin"],
        inputs["wq"], inputs["wk"], inputs["wv"], inputs["wo"],
    )
    res = bass_utils.run_bass_kernel_spmd(
        nc, in_maps, list(range(N_CORES)), trace=trace, **spmd_kwargs
    )
    out = np.zeros((B, S, D), dtype=np.float32)
    for c in range(N_CORES):
        out[c // CPB] += np.asarray(res.results[c]["y"]).astype(np.float32)
    return out, res


def kernel(**inputs):
    out, _ = run(inputs, trace=False)
    return out


def simulate_core(core_idx, inputs):
    """CoreSim-validate a single core's program; returns its partial y."""
    from concourse.bass_interp import CoreSim

    nc = build_program()
    in_maps = make_core_inputs(
        inputs["x"], inputs["freqs_cos"], inputs["freqs_sin"],
        inputs["wq"], inputs["wk"], inputs["wv"], inputs["wo"],
    )
    sim = CoreSim(nc)
    for name, arr in in_maps[core_idx].items():
        sim.tensor(name)[:] = arr
    sim.simulate()
    return np.array(sim.tensor("y"))


# revision 33
# speedup vs baseline: 1.0346x; 1.0346x over previous
"""Trainium2 Bass kernel for nn_Attention (dense transformer block):
y = Attention(RoPE(x@wqT), RoPE(x@wkT), x@wvT, causal) @ woT

Sharding: 8 cores = 2 batches x 4 head-groups (tensor-parallel heads,
data-parallel batch).  Each core handles one batch and 4 of the 16 heads
(512 of the 2048 channels): column-shard of wq/wk/wv, row-shard of wo.
Each core emits a full-shape [S, D] partial of y; the host sums the 4
partials per batch.

Fused chunk pipeline (per core, SPMD): for each 512-row seq chunk j,
  proj(j):  q/k (+RoPE on DVE) and v for chunk j from ONE pass over x
            tiles (v reuses the same SBUF x tiles as stationary
            operands — x is streamed from DRAM exactly once, in bf16)
  attn(j):  causal attention for all 4 heads over keys 0..j, transposed
            scores (sT[sk,sq] = kT.T @ qT, fp32r), exp on ACT -> bf16
            probs, causal mask + denominator tile-accumulation as DVE
            elementwise ops (one ones-matmul per head reduces the
            accumulator over partitions), software-pipelined with a
            3-tile lookahead so the PE never waits on exp
  wo(j):    row-parallel wo partial, nn-inner loop so each stationary
            out_h slice is loaded once for 4 matmuls (LDWEIGHTS
            amortization); psum borrowed from the idle score pool;
            interleaved with proj(j+1)'s v chains
Chunk j+1's x tiles prefetch during attn(j); all weights stay resident.
Everything is sized against HW-calibrated costs: matmul ~ N/2.4GHz +
~90ns dispatch + ~50ns on stationary rotation; per-instruction count,
not FLOPs, is the limiting budget beyond the 290us streaming floor.
"""

import os
import sys

import numpy as np

for _p in ("/opt/trn_rl_repo", "/root/.axon_site/_ro/trn_rl_repo"):
    if os.path.isdir(_p) and _p not in sys.path:
        sys.path.insert(0, _p)

import concourse.bass as bass
import concourse.tile as tile
from concourse import bacc
from concourse import mybir
from concourse import bass_utils

B, S, D, H = 2, 2048, 2048, 16
HD = 128                 # head dim
HPC = 4                  # heads per core
CPB = 4                  # cores per batch
N_CORES = 8
NK = D // 128            # 16 contraction chunks
NSQ = S // 512           # 4 sq chunks of 512
NSK = S // 128           # 16 sk tiles of 128
SCALE = float(1.0 / np.sqrt(np.float32(HD)))

F32 = mybir.dt.float32
F32R = mybir.dt.float32r
BF16 = mybir.dt.bfloat16

EXP = mybir.ActivationFunctionType.Exp
SWAP_MASK = [i ^ 1 for i in range(32)]

# diag-tile mask pattern m: columns < 128*m fully masked; keep score width
# >= 256 so the fp32r moving operand stays at full rate.  The bf16 pv/den
# side has no width constraint and can skip all fully-masked columns.
DIAG_OFF = {0: 0, 1: 128, 2: 256, 3: 256}
PV_OFF = {0: 0, 1: 128, 2: 256, 3: 384}
LOOKAHEAD = 3


def _emit(tc):
    nc = tc.nc

    xT = nc.dram_tensor("xT", [D, S], BF16, kind="ExternalInput").ap()
    wqT = nc.dram_tensor("wqT", [D, HPC * HD], BF16, kind="ExternalInput").ap()
    wkT = nc.dram_tensor("wkT", [D, HPC * HD], BF16, kind="ExternalInput").ap()
    wvT = nc.dram_tensor("wvT", [D, HPC * HD], BF16, kind="ExternalInput").ap()
    woT = nc.dram_tensor("woT", [HPC * HD, D], BF16, kind="ExternalInput").ap()
    cosq = nc.dram_tensor("cosq", [HD, S], F32, kind="ExternalInput").ap()
    sinq = nc.dram_tensor("sinq", [HD, S], F32, kind="ExternalInput").ap()
    mask01 = nc.dram_tensor("mask01", [128, 4, 512], BF16, kind="ExternalInput").ap()
    onesd = nc.dram_tensor("onesd", [128, 128], F32R, kind="ExternalInput").ap()
    y = nc.dram_tensor("y", [S, D], BF16, kind="ExternalOutput").ap()

    ld = nc.sync        # all loads on the SP HWDGE queue
    st = nc.scalar      # y stores on the ACT HWDGE queue

    # ---- SBUF pools (all live for the whole kernel)
    consts = tc.alloc_tile_pool(name="consts", bufs=1)
    mask_sb = consts.tile([128, 4, 512], BF16, name="mask_sb")
    ones_sb = consts.tile([128, 128], F32R, name="ones_sb")

    wpool = tc.alloc_tile_pool(name="wpool", bufs=1)
    wq_sb = wpool.tile([128, NK, HPC * HD], BF16, name="wq_sb")
    wk_sb = wpool.tile([128, NK, HPC * HD], BF16, name="wk_sb")
    wv_sb = wpool.tile([128, NK, HPC * HD], BF16, name="wv_sb")
    wo_sb = wpool.tile([128, HPC, D], BF16, name="wo_sb")

    kpool = tc.alloc_tile_pool(name="kpool", bufs=1)
    kT = [[kpool.tile([128, 512], F32R, name=f"kT{h}_{j}") for j in range(NSQ)]
          for h in range(HPC)]
    vpool = tc.alloc_tile_pool(name="vpool", bufs=1)
    v_sb = [vpool.tile([128, HPC * HD], BF16, name=f"v{m}") for m in range(NSK)]

    qpool = tc.alloc_tile_pool(name="qpool", bufs=1)
    ropec = tc.alloc_tile_pool(name="ropec", bufs=2)
    xpool = tc.alloc_tile_pool(name="xpool", bufs=2)
    tpool = tc.alloc_tile_pool(name="tpool", bufs=2)
    ptpool = tc.alloc_tile_pool(name="ptpool", bufs=8)
    dpool = tc.alloc_tile_pool(name="dpool", bufs=3)
    opool = tc.alloc_tile_pool(name="opool", bufs=1)
    rpool = tc.alloc_tile_pool(name="rpool", bufs=2)
    ypool = tc.alloc_tile_pool(name="ypool", bufs=2)

    pg = tc.alloc_tile_pool(name="pg", bufs=2, space="PSUM")
    pss = tc.alloc_tile_pool(name="pss", bufs=LOOKAHEAD + 1, space="PSUM")
    pspv = tc.alloc_tile_pool(name="pspv", bufs=2, space="PSUM")

    # ---- prologue DMAs; xs/wq split in halves so the first chain starts
    # after ~1/4 of the lead-in bytes.  Loads split over both HWDGE queues.
    def load_x(j, nsplit=2):
        xt = xpool.tile([128, NK, 512], BF16, name=f"xs{j}", tag="xs")
        src = xT[:, 512 * j:512 * (j + 1)].rearrange("(kt p) c -> p kt c", p=128)
        step = NK // nsplit
        for q in range(nsplit):
            ld.dma_start(
                out=xt[:, step * q:step * (q + 1), :],
                in_=src[:, step * q:step * (q + 1), :],
            )
        return xt

    def load_rope(j):
        ct = ropec.tile([128, 512], F32, name=f"cos{j}", tag="cos")
        st.dma_start(out=ct, in_=cosq[:, 512 * j:512 * (j + 1)])
        sn = ropec.tile([128, 512], F32, name=f"sin{j}", tag="sin")
        st.dma_start(out=sn, in_=sinq[:, 512 * j:512 * (j + 1)])
        return ct, sn

    xs = load_x(0, nsplit=4)
    wq_src = wqT.rearrange("(kt p) c -> p kt c", p=128)
    for q in range(4):
        st.dma_start(
            out=wq_sb[:, 4 * q:4 * (q + 1), :], in_=wq_src[:, 4 * q:4 * (q + 1), :]
        )
    ld.dma_start(out=wk_sb, in_=wkT.rearrange("(kt p) c -> p kt c", p=128))
    ct0 = ropec.tile([128, 512], F32, name="cos0", tag="cos")
    ld.dma_start(out=ct0, in_=cosq[:, 0:512])
    sn0 = ropec.tile([128, 512], F32, name="sin0", tag="sin")
    ld.dma_start(out=sn0, in_=sinq[:, 0:512])
    cs0 = (ct0, sn0)
    ld.dma_start(out=wv_sb, in_=wvT.rearrange("(kt p) c -> p kt c", p=128))
    st.dma_start(out=mask_sb, in_=mask01)
    st.dma_start(out=ones_sb, in_=onesd)
    ld.dma_start(out=wo_sb, in_=woT.rearrange("(h p) d -> p h d", p=128))

    qT = [None] * HPC    # per-chunk q tiles, rewritten each chunk
    out_h = [None] * HPC

    def proj_chains(j, xs, cos_sb, sin_sb):
        """Closures: 8 q/k chains (+RoPE drain on DVE), 4 v chains (ACT
        drain).  The psum is freed by the 2nd DVE op (t2) for q/k, and by
        the single ACT copy for v."""
        chains = []

        def qk_chain(which, w_sb, h):
            def emit():
                acc = pg.tile([128, 512], F32, name=f"a{which}{j}_{h}", tag="pg")
                for k in range(NK):
                    nc.tensor.matmul(
                        acc, w_sb[:, k, 128 * h:128 * (h + 1)], xs[:, k, :],
                        start=(k == 0), stop=(k == NK - 1),
                    )
                if which == "q":
                    dst = qpool.tile([128, 512], F32R, name=f"qT{h}_{j}", tag=f"q{h}")
                    qT[h] = dst
                else:
                    dst = kT[h][j]
                shuf = tpool.tile([128, 512], F32, name=f"sh{which}{j}_{h}", tag="shuf")
                nc.vector.stream_shuffle(shuf, acc, SWAP_MASK)
                t2 = tpool.tile([128, 512], F32, name=f"t2{which}{j}_{h}", tag="t1")
                nc.vector.tensor_mul(t2, acc, cos_sb)
                t1 = tpool.tile([128, 512], F32, name=f"t1{which}{j}_{h}", tag="shuf")
                nc.vector.tensor_mul(t1, shuf, sin_sb)
                nc.vector.tensor_add(dst, t1, t2)
            return emit

        def v_chain(m):
            def emit():
                acc = pg.tile([128, HPC * HD], F32, name=f"av{j}_{m}", tag="pg")
                for k in range(NK):
                    nc.tensor.matmul(
                        acc, xs[:, k, 128 * m:128 * (m + 1)], wv_sb[:, k, :],
                        start=(k == 0), stop=(k == NK - 1),
                    )
                nc.scalar.copy(v_sb[4 * j + m], acc)
            return emit

        for which, w_sb in (("q", wq_sb), ("k", wk_sb)):
            for h in range(HPC):
                chains.append(qk_chain(which, w_sb, h))
        for m in range(4):
            chains.append(v_chain(m))
        return chains

    def emit_attn(j):
        """Causal attention for all heads over keys 0..4j+3, pipelined.
        Denominator = running sum of prob tiles on the DVE (per-head SBUF
        accumulator); causal mask = DVE multiply after the exp."""
        nsk = 4 * j + 4
        # flat tile list: per head, off-diagonal tiles first, then diagonal
        tiles = []
        for h in range(HPC):
            order = list(range(0, 4 * j)) + list(range(4 * j, nsk))
            for pos, i in enumerate(order):
                tiles.append((h, i, pos, pos == nsk - 1))
        n = len(tiles)
        st_of = {}   # tile idx -> (pt, off)
        pv_of = {}
        dacc_of = {}

        def emit_score(t):
            h, i, pos, last = tiles[t]
            off = DIAG_OFF[i - 4 * j] if i >= 4 * j else 0
            cs = slice(off, 512)
            s_ps = pss.tile([128, 512], F32, name=f"s{j}_{t}", tag="s")
            nc.tensor.matmul(
                s_ps[:, cs], kT[h][i // 4][:, 128 * (i % 4):128 * (i % 4 + 1)],
                qT[h][:, cs], start=True, stop=True,
            )
            pt = ptpool.tile([128, 512], BF16, name=f"p{j}_{t}", tag="pt")
            nc.scalar.activation(pt[:, cs], s_ps[:, cs], EXP, bias=0.0, scale=SCALE)
            if i >= 4 * j:
                nc.vector.tensor_mul(
                    pt[:, cs], pt[:, cs], mask_sb[:, i - 4 * j, cs]
                )
            # denominator running sum; heads 0,2 on DVE, heads 1,3 on the
            # otherwise-idle GpSimd (pos 0 tile is always full-width)
            off2 = PV_OFF[i - 4 * j] if i >= 4 * j else 0
            cs2 = slice(off2, 512)
            deng = nc.vector if h % 2 == 0 else nc.gpsimd
            if pos == 0:
                dacc_of[h] = dpool.tile([128, 512], F32R, name=f"da{j}_{h}", tag="dacc")
                deng.tensor_copy(out=dacc_of[h], in_=pt)
            else:
                deng.tensor_add(dacc_of[h][:, cs2], dacc_of[h][:, cs2], pt[:, cs2])
            st_of[t] = (pt, off2)

        def emit_pv(t):
            h, i, pos, last = tiles[t]
            pt, off = st_of.pop(t)
            cs = slice(off, 512)
            if pos == 0:
                pv_of[h] = pspv.tile([128, 512], F32, name=f"pv{j}_{h}", tag="pv")
            nc.tensor.matmul(
                pv_of[h][:, cs], v_sb[i][:, 128 * h:128 * (h + 1)], pt[:, cs],
                start=(pos == 0), stop=last,
            )
            if last:
                den_ps = pss.tile([128, 512], F32, name=f"dn{j}_{h}", tag="s")
                nc.tensor.matmul(den_ps, ones_sb, dacc_of[h], start=True, stop=True)
                recip = rpool.tile([128, 512], F32, name=f"rc{j}_{h}", tag="recip")
                nc.vector.reciprocal(recip, den_ps)
                oh = opool.tile([128, 512], BF16, name=f"oh{j}_{h}", tag=f"o{h}")
                nc.vector.tensor_mul(oh, pv_of[h], recip)
                out_h[h] = oh

        for t in range(min(LOOKAHEAD, n)):
            emit_score(t)
        for t in range(n):
            if t + LOOKAHEAD < n:
                emit_score(t + LOOKAHEAD)
            emit_pv(t)

    def wo_groups(j):
        """Closures: 4 wo row-blocks.  Within a block the stationary
        out_h[h] slice is loaded once and streams all 4 col-slices (nn
        inner), accumulating into 4 psum banks borrowed from the (idle
        during wo) score pool."""
        groups = []

        def block(tt):
            def emit():
                ys = ypool.tile([128, D], BF16, name=f"ys{j}_{tt}", tag="ys")
                accs = [
                    pss.tile([128, 512], F32, name=f"wy{j}_{tt}_{nn}", tag="s")
                    for nn in range(4)
                ]
                for h in range(HPC):
                    for nn in range(4):
                        nc.tensor.matmul(
                            accs[nn],
                            out_h[h][:, 128 * tt:128 * (tt + 1)],
                            wo_sb[:, h, 512 * nn:512 * (nn + 1)],
                            start=(h == 0), stop=(h == HPC - 1),
                        )
                for nn in range(4):
                    dst = ys[:, 512 * nn:512 * (nn + 1)]
                    if nn % 2 == 0:
                        nc.vector.tensor_copy(out=dst, in_=accs[nn])
                    else:
                        nc.scalar.copy(dst, accs[nn])
                row = 512 * j + 128 * tt
                st.dma_start(out=y[row:row + 128, :], in_=ys)
            return emit

        for tt in range(4):
            groups.append(block(tt))
        return groups

    # Steady state per chunk j: attn(j) -> proj(j+1) q/k chains (their long
    # RoPE drains never sit between two short psum groups) -> v chains of
    # proj(j+1) interleaved 1:4 with wo(j) groups (v's ACT-copy drain frees
    # its psum fast enough for the interleave).
    for c in proj_chains(0, xs, *cs0):
        c()
    for j in range(NSQ):
        if j + 1 < NSQ:
            xs_next = load_x(j + 1)
            cs_next = load_rope(j + 1)
        emit_attn(j)
        wo = wo_groups(j)
        if j + 1 < NSQ:
            chains = proj_chains(j + 1, xs_next, *cs_next)
            for c in chains[:8]:      # q/k chains
                c()
            for m in range(4):        # v chains, wo blocks woven between
                chains[8 + m]()
                wo[m]()
        else:
            for g in wo:
                g()

    for p in (pspv, pss, pg, ypool, rpool, opool, dpool, ptpool, tpool,
              xpool, ropec, qpool, vpool, kpool, wpool, consts):
        p.release()


_PROGRAM = None


def build_program():
    global _PROGRAM
    if _PROGRAM is None:
        nc = bacc.Bacc("TRN2", target_bir_lowering=False, debug=False)
        with tile.TileContext(nc) as tc:
            _emit(tc)
        nc.compile()
        _PROGRAM = nc
    return _PROGRAM


def make_core_inputs(x, freqs_cos, freqs_sin, wq, wk, wv, wo):
    """Host-side sharding: returns list of 8 per-core input dicts."""
    import ml_dtypes

    bf16 = ml_dtypes.bfloat16
    x = np.asarray(x, dtype=np.float32)
    freqs_cos = np.asarray(freqs_cos, dtype=np.float32)
    freqs_sin = np.asarray(freqs_sin, dtype=np.float32)
    wq = np.asarray(wq, dtype=np.float32)
    wk = np.asarray(wk, dtype=np.float32)
    wv = np.asarray(wv, dtype=np.float32)
    wo = np.asarray(wo, dtype=np.float32)

    cosq = np.ascontiguousarray(np.repeat(freqs_cos.T, 2, axis=0))  # [128, S]
    sinq = np.ascontiguousarray(np.repeat(freqs_sin.T, 2, axis=0))
    sinq[0::2, :] *= -1.0  # even rows: -sin; odd rows: +sin

    skl = np.arange(128)[:, None]
    sql = np.arange(512)[None, :]
    mask01 = np.stack(
        [(128 * m + skl <= sql).astype(bf16) for m in range(4)], axis=1
    )  # [128, 4, 512]

    xTs = [np.ascontiguousarray(x[b].T).astype(bf16) for b in range(B)]
    in_maps = []
    for c in range(N_CORES):
        b, g = divmod(c, CPB)
        hsl = slice(512 * g, 512 * (g + 1))
        in_maps.append(
            {
                "xT": xTs[b],
                "wqT": np.ascontiguousarray(wq[hsl, :].T).astype(bf16),
                "wkT": np.ascontiguousarray(wk[hsl, :].T).astype(bf16),
                "wvT": np.ascontiguousarray(wv[hsl, :].T).astype(bf16),
                "woT": np.ascontiguousarray(wo[:, hsl].T).astype(bf16),
                "cosq": cosq,
                "sinq": sinq,
                "mask01": mask01,
                "onesd": np.ones((128, 128), dtype=np.float32),
            }
        )
    return in_maps


def run(inputs, trace=False, **spmd_kwargs):
    """Run the SPMD kernel on 8 cores.  Returns (y_full, BassKernelResults)."""
    nc = build_program()
    in_maps = make_core_inputs(
        inputs["x"], inputs["freqs_cos"], inputs["freqs_sin"],
        inputs["wq"], inputs["wk"], inputs["wv"], inputs["wo"],
    )
    res = bass_utils.run_bass_kernel_spmd(
        nc, in_maps, list(range(N_CORES)), trace=trace, **spmd_kwargs
    )
    out = np.zeros((B, S, D), dtype=np.float32)
    for c in range(N_CORES):
        out[c // CPB] += np.asarray(res.results[c]["y"]).astype(np.float32)
    return out, res


def kernel(**inputs):
    out, _ = run(inputs, trace=False)
    return out


def simulate_core(core_idx, inputs):
    """CoreSim-validate a single core's program; returns its partial y."""
    from concourse.bass_interp import CoreSim

    nc = build_program()
    in_maps = make_core_inputs(
        inputs["x"], inputs["freqs_cos"], inputs["freqs_sin"],
        inputs["wq"], inputs["wk"], inputs["wv"], inputs["wo"],
    )
    sim = CoreSim(nc)
    for name, arr in in_maps[core_idx].items():
        sim.tensor(name)[:] = arr
    sim.simulate()
    return np.array(sim.tensor("y"))


# revision 34
# speedup vs baseline: 1.0422x; 1.0073x over previous
"""Trainium2 Bass kernel for nn_Attention (dense transformer block):
y = Attention(RoPE(x@wqT), RoPE(x@wkT), x@wvT, causal) @ woT

Sharding: 8 cores = 2 batches x 4 head-groups (tensor-parallel heads,
data-parallel batch).  Each core handles one batch and 4 of the 16 heads
(512 of the 2048 channels): column-shard of wq/wk/wv, row-shard of wo.
Each core emits a full-shape [S, D] partial of y; the host sums the 4
partials per batch.

Fused chunk pipeline (per core, SPMD): for each 512-row seq chunk j,
  proj(j):  q/k (+RoPE on DVE) and v for chunk j from ONE pass over x
            tiles (v reuses the same SBUF x tiles as stationary
            operands — x is streamed from DRAM exactly once, in bf16)
  attn(j):  causal attention for all 4 heads over keys 0..j, transposed
            scores (sT[sk,sq] = kT.T @ qT, fp32r), exp on ACT -> bf16
            probs, causal mask + denominator tile-accumulation as DVE
            elementwise ops (one ones-matmul per head reduces the
            accumulator over partitions), software-pipelined with a
            3-tile lookahead so the PE never waits on exp
  wo(j):    row-parallel wo partial, nn-inner loop so each stationary
            out_h slice is loaded once for 4 matmuls (LDWEIGHTS
            amortization); psum borrowed from the idle score pool;
            interleaved with proj(j+1)'s v chains
Chunk j+1's x tiles prefetch during attn(j); all weights stay resident.
Everything is sized against HW-calibrated costs: matmul ~ N/2.4GHz +
~90ns dispatch + ~50ns on stationary rotation; per-instruction count,
not FLOPs, is the limiting budget beyond the 290us streaming floor.
"""

import os
import sys

import numpy as np

for _p in ("/opt/trn_rl_repo", "/root/.axon_site/_ro/trn_rl_repo"):
    if os.path.isdir(_p) and _p not in sys.path:
        sys.path.insert(0, _p)

import concourse.bass as bass
import concourse.tile as tile
from concourse import bacc
from concourse import mybir
from concourse import bass_utils

B, S, D, H = 2, 2048, 2048, 16
HD = 128                 # head dim
HPC = 4                  # heads per core
CPB = 4                  # cores per batch
N_CORES = 8
NK = D // 128            # 16 contraction chunks
NSQ = S // 512           # 4 sq chunks of 512
NSK = S // 128           # 16 sk tiles of 128
SCALE = float(1.0 / np.sqrt(np.float32(HD)))

F32 = mybir.dt.float32
F32R = mybir.dt.float32r
BF16 = mybir.dt.bfloat16

EXP = mybir.ActivationFunctionType.Exp
SWAP_MASK = [i ^ 1 for i in range(32)]

# diag-tile mask pattern m: columns < 128*m fully masked; keep score width
# >= 256 so the fp32r moving operand stays at full rate.  The bf16 pv/den
# side has no width constraint and can skip all fully-masked columns.
DIAG_OFF = {0: 0, 1: 128, 2: 256, 3: 256}
PV_OFF = {0: 0, 1: 128, 2: 256, 3: 384}
LOOKAHEAD = 3


def _emit(tc):
    nc = tc.nc

    xT = nc.dram_tensor("xT", [D, S], BF16, kind="ExternalInput").ap()
    wqT = nc.dram_tensor("wqT", [D, HPC * HD], BF16, kind="ExternalInput").ap()
    wkT = nc.dram_tensor("wkT", [D, HPC * HD], BF16, kind="ExternalInput").ap()
    wvT = nc.dram_tensor("wvT", [D, HPC * HD], BF16, kind="ExternalInput").ap()
    woT = nc.dram_tensor("woT", [HPC * HD, D], BF16, kind="ExternalInput").ap()
    cosq = nc.dram_tensor("cosq", [HD, S], F32, kind="ExternalInput").ap()
    sinq = nc.dram_tensor("sinq", [HD, S], F32, kind="ExternalInput").ap()
    mask01 = nc.dram_tensor("mask01", [128, 4, 512], BF16, kind="ExternalInput").ap()
    onesd = nc.dram_tensor("onesd", [128, 128], F32R, kind="ExternalInput").ap()
    y = nc.dram_tensor("y", [S, D], BF16, kind="ExternalOutput").ap()

    ld = nc.sync        # all loads on the SP HWDGE queue
    st = nc.scalar      # y stores on the ACT HWDGE queue

    # ---- SBUF pools (all live for the whole kernel)
    consts = tc.alloc_tile_pool(name="consts", bufs=1)
    mask_sb = consts.tile([128, 4, 512], BF16, name="mask_sb")
    ones_sb = consts.tile([128, 128], F32R, name="ones_sb")

    wpool = tc.alloc_tile_pool(name="wpool", bufs=1)
    wq_sb = wpool.tile([128, NK, HPC * HD], BF16, name="wq_sb")
    wk_sb = wpool.tile([128, NK, HPC * HD], BF16, name="wk_sb")
    wv_sb = wpool.tile([128, NK, HPC * HD], BF16, name="wv_sb")
    wo_sb = wpool.tile([128, HPC, D], BF16, name="wo_sb")

    kpool = tc.alloc_tile_pool(name="kpool", bufs=1)
    kT = [[kpool.tile([128, 512], F32R, name=f"kT{h}_{j}") for j in range(NSQ)]
          for h in range(HPC)]
    vpool = tc.alloc_tile_pool(name="vpool", bufs=1)
    v_sb = [vpool.tile([128, HPC * HD], BF16, name=f"v{m}") for m in range(NSK)]

    qpool = tc.alloc_tile_pool(name="qpool", bufs=1)
    ropec = tc.alloc_tile_pool(name="ropec", bufs=2)
    xpool = tc.alloc_tile_pool(name="xpool", bufs=2)
    tpool = tc.alloc_tile_pool(name="tpool", bufs=2)
    ptpool = tc.alloc_tile_pool(name="ptpool", bufs=8)
    dpool = tc.alloc_tile_pool(name="dpool", bufs=3)
    opool = tc.alloc_tile_pool(name="opool", bufs=1)
    rpool = tc.alloc_tile_pool(name="rpool", bufs=2)
    ypool = tc.alloc_tile_pool(name="ypool", bufs=2)

    pg = tc.alloc_tile_pool(name="pg", bufs=2, space="PSUM")
    pss = tc.alloc_tile_pool(name="pss", bufs=LOOKAHEAD + 1, space="PSUM")
    pspv = tc.alloc_tile_pool(name="pspv", bufs=2, space="PSUM")

    # ---- prologue DMAs; xs/wq split in halves so the first chain starts
    # after ~1/4 of the lead-in bytes.  Loads split over both HWDGE queues.
    def load_x(j, nsplit=2):
        xt = xpool.tile([128, NK, 512], BF16, name=f"xs{j}", tag="xs")
        src = xT[:, 512 * j:512 * (j + 1)].rearrange("(kt p) c -> p kt c", p=128)
        step = NK // nsplit
        for q in range(nsplit):
            ld.dma_start(
                out=xt[:, step * q:step * (q + 1), :],
                in_=src[:, step * q:step * (q + 1), :],
            )
        return xt

    def load_rope(j):
        ct = ropec.tile([128, 512], F32, name=f"cos{j}", tag="cos")
        st.dma_start(out=ct, in_=cosq[:, 512 * j:512 * (j + 1)])
        sn = ropec.tile([128, 512], F32, name=f"sin{j}", tag="sin")
        st.dma_start(out=sn, in_=sinq[:, 512 * j:512 * (j + 1)])
        return ct, sn

    xs = load_x(0, nsplit=4)
    wq_src = wqT.rearrange("(kt p) c -> p kt c", p=128)
    for q in range(4):
        st.dma_start(
            out=wq_sb[:, 4 * q:4 * (q + 1), :], in_=wq_src[:, 4 * q:4 * (q + 1), :]
        )
    ld.dma_start(out=wk_sb, in_=wkT.rearrange("(kt p) c -> p kt c", p=128))
    ct0 = ropec.tile([128, 512], F32, name="cos0", tag="cos")
    ld.dma_start(out=ct0, in_=cosq[:, 0:512])
    sn0 = ropec.tile([128, 512], F32, name="sin0", tag="sin")
    ld.dma_start(out=sn0, in_=sinq[:, 0:512])
    cs0 = (ct0, sn0)
    ld.dma_start(out=wv_sb, in_=wvT.rearrange("(kt p) c -> p kt c", p=128))
    st.dma_start(out=mask_sb, in_=mask01)
    st.dma_start(out=ones_sb, in_=onesd)
    ld.dma_start(out=wo_sb, in_=woT.rearrange("(h p) d -> p h d", p=128))

    qT = [None] * HPC    # per-chunk q tiles, rewritten each chunk
    out_h = [None] * HPC

    def proj_chains(j, xs, cos_sb, sin_sb):
        """Closures: 8 q/k chains (+RoPE drain on DVE), 4 v chains (ACT
        drain).  The psum is freed by the 2nd DVE op (t2) for q/k, and by
        the single ACT copy for v."""
        chains = []

        def qk_chain(which, w_sb, h):
            def emit():
                acc = pg.tile([128, 512], F32, name=f"a{which}{j}_{h}", tag="pg")
                for k in range(NK):
                    nc.tensor.matmul(
                        acc, w_sb[:, k, 128 * h:128 * (h + 1)], xs[:, k, :],
                        start=(k == 0), stop=(k == NK - 1),
                    )
                if which == "q":
                    dst = qpool.tile([128, 512], F32R, name=f"qT{h}_{j}", tag=f"q{h}")
                    qT[h] = dst
                else:
                    dst = kT[h][j]
                shuf = tpool.tile([128, 512], F32, name=f"sh{which}{j}_{h}", tag="shuf")
                nc.vector.stream_shuffle(shuf, acc, SWAP_MASK)
                t2 = tpool.tile([128, 512], F32, name=f"t2{which}{j}_{h}", tag="t1")
                nc.vector.tensor_mul(t2, acc, cos_sb)
                t1 = tpool.tile([128, 512], F32, name=f"t1{which}{j}_{h}", tag="shuf")
                nc.vector.tensor_mul(t1, shuf, sin_sb)
                nc.vector.tensor_add(dst, t1, t2)
            return emit

        def v_chain(m):
            def emit():
                acc = pg.tile([128, HPC * HD], F32, name=f"av{j}_{m}", tag="pg")
                for k in range(NK):
                    nc.tensor.matmul(
                        acc, xs[:, k, 128 * m:128 * (m + 1)], wv_sb[:, k, :],
                        start=(k == 0), stop=(k == NK - 1),
                    )
                nc.scalar.copy(v_sb[4 * j + m], acc)
            return emit

        for which, w_sb in (("q", wq_sb), ("k", wk_sb)):
            for h in range(HPC):
                chains.append(qk_chain(which, w_sb, h))
        for m in range(4):
            chains.append(v_chain(m))
        return chains

    def emit_attn(j):
        """Causal attention for all heads over keys 0..4j+3, pipelined.
        Denominator = running sum of prob tiles on the DVE (per-head SBUF
        accumulator); causal mask = DVE multiply after the exp."""
        nsk = 4 * j + 4
        # flat tile list: per head, off-diagonal tiles first, then diagonal
        tiles = []
        for h in range(HPC):
            order = list(range(0, 4 * j)) + list(range(4 * j, nsk))
            for pos, i in enumerate(order):
                tiles.append((h, i, pos, pos == nsk - 1))
        n = len(tiles)
        st_of = {}   # tile idx -> (pt, off)
        pv_of = {}
        dacc_of = {}

        def emit_score(t):
            h, i, pos, last = tiles[t]
            off = DIAG_OFF[i - 4 * j] if i >= 4 * j else 0
            cs = slice(off, 512)
            s_ps = pss.tile([128, 512], F32, name=f"s{j}_{t}", tag="s")
            nc.tensor.matmul(
                s_ps[:, cs], kT[h][i // 4][:, 128 * (i % 4):128 * (i % 4 + 1)],
                qT[h][:, cs], start=True, stop=True,
            )
            pt = ptpool.tile([128, 512], BF16, name=f"p{j}_{t}", tag="pt")
            nc.scalar.activation(pt[:, cs], s_ps[:, cs], EXP, bias=0.0, scale=SCALE)
            if i >= 4 * j:
                # the mask is non-trivial only in the 128-col stair band
                # [128m, 128m+128); columns right of it are all-ones
                m = i - 4 * j
                ms = slice(128 * m, 128 * (m + 1))
                nc.vector.tensor_mul(pt[:, ms], pt[:, ms], mask_sb[:, m, ms])
            # denominator running sum; heads 0,2 on DVE, heads 1,3 on the
            # otherwise-idle GpSimd (pos 0 tile is always full-width)
            off2 = PV_OFF[i - 4 * j] if i >= 4 * j else 0
            cs2 = slice(off2, 512)
            deng = nc.vector if h % 2 == 0 else nc.gpsimd
            if pos == 0:
                dacc_of[h] = dpool.tile([128, 512], F32R, name=f"da{j}_{h}", tag="dacc")
                deng.tensor_copy(out=dacc_of[h], in_=pt)
            else:
                deng.tensor_add(dacc_of[h][:, cs2], dacc_of[h][:, cs2], pt[:, cs2])
            st_of[t] = (pt, off2)

        def emit_pv(t):
            h, i, pos, last = tiles[t]
            pt, off = st_of.pop(t)
            cs = slice(off, 512)
            if pos == 0:
                pv_of[h] = pspv.tile([128, 512], F32, name=f"pv{j}_{h}", tag="pv")
            nc.tensor.matmul(
                pv_of[h][:, cs], v_sb[i][:, 128 * h:128 * (h + 1)], pt[:, cs],
                start=(pos == 0), stop=last,
            )
            if last:
                den_ps = pss.tile([128, 512], F32, name=f"dn{j}_{h}", tag="s")
                nc.tensor.matmul(den_ps, ones_sb, dacc_of[h], start=True, stop=True)
                recip = rpool.tile([128, 512], F32, name=f"rc{j}_{h}", tag="recip")
                nc.vector.reciprocal(recip, den_ps)
                oh = opool.tile([128, 512], BF16, name=f"oh{j}_{h}", tag=f"o{h}")
                nc.vector.tensor_mul(oh, pv_of[h], recip)
                out_h[h] = oh

        for t in range(min(LOOKAHEAD, n)):
            emit_score(t)
        for t in range(n):
            if t + LOOKAHEAD < n:
                emit_score(t + LOOKAHEAD)
            emit_pv(t)

    def wo_groups(j):
        """Closures: 4 wo row-blocks.  Within a block the stationary
        out_h[h] slice is loaded once and streams all 4 col-slices (nn
        inner), accumulating into 4 psum banks borrowed from the (idle
        during wo) score pool."""
        groups = []

        def block(tt):
            def emit():
                ys = ypool.tile([128, D], BF16, name=f"ys{j}_{tt}", tag="ys")
                accs = [
                    pss.tile([128, 512], F32, name=f"wy{j}_{tt}_{nn}", tag="s")
                    for nn in range(4)
                ]
                for h in range(HPC):
                    for nn in range(4):
                        nc.tensor.matmul(
                            accs[nn],
                            out_h[h][:, 128 * tt:128 * (tt + 1)],
                            wo_sb[:, h, 512 * nn:512 * (nn + 1)],
                            start=(h == 0), stop=(h == HPC - 1),
                        )
                for nn in range(4):
                    dst = ys[:, 512 * nn:512 * (nn + 1)]
                    if nn % 2 == 0:
                        nc.vector.tensor_copy(out=dst, in_=accs[nn])
                    else:
                        nc.scalar.copy(dst, accs[nn])
                row = 512 * j + 128 * tt
                st.dma_start(out=y[row:row + 128, :], in_=ys)
            return emit

        for tt in range(4):
            groups.append(block(tt))
        return groups

    # Steady state per chunk j: attn(j) -> proj(j+1) q/k chains (their long
    # RoPE drains never sit between two short psum groups) -> v chains of
    # proj(j+1) interleaved 1:4 with wo(j) groups (v's ACT-copy drain frees
    # its psum fast enough for the interleave).
    for c in proj_chains(0, xs, *cs0):
        c()
    for j in range(NSQ):
        if j + 1 < NSQ:
            xs_next = load_x(j + 1)
            cs_next = load_rope(j + 1)
        emit_attn(j)
        wo = wo_groups(j)
        if j + 1 < NSQ:
            chains = proj_chains(j + 1, xs_next, *cs_next)
            for c in chains[:8]:      # q/k chains
                c()
            for m in range(4):        # v chains, wo blocks woven between
                chains[8 + m]()
                wo[m]()
        else:
            for g in wo:
                g()

    for p in (pspv, pss, pg, ypool, rpool, opool, dpool, ptpool, tpool,
              xpool, ropec, qpool, vpool, kpool, wpool, consts):
        p.release()


_PROGRAM = None


def build_program():
    global _PROGRAM
    if _PROGRAM is None:
        nc = bacc.Bacc("TRN2", target_bir_lowering=False, debug=False)
        with tile.TileContext(nc) as tc:
            _emit(tc)
        nc.compile()
        _PROGRAM = nc
    return _PROGRAM


def make_core_inputs(x, freqs_cos, freqs_sin, wq, wk, wv, wo):
    """Host-side sharding: returns list of 8 per-core input dicts."""
    import ml_dtypes

    bf16 = ml_dtypes.bfloat16
    x = np.asarray(x, dtype=np.float32)
    freqs_cos = np.asarray(freqs_cos, dtype=np.float32)
    freqs_sin = np.asarray(freqs_sin, dtype=np.float32)
    wq = np.asarray(wq, dtype=np.float32)
    wk = np.asarray(wk, dtype=np.float32)
    wv = np.asarray(wv, dtype=np.float32)
    wo = np.asarray(wo, dtype=np.float32)

    cosq = np.ascontiguousarray(np.repeat(freqs_cos.T, 2, axis=0))  # [128, S]
    sinq = np.ascontiguousarray(np.repeat(freqs_sin.T, 2, axis=0))
    sinq[0::2, :] *= -1.0  # even rows: -sin; odd rows: +sin

    skl = np.arange(128)[:, None]
    sql = np.arange(512)[None, :]
    mask01 = np.stack(
        [(128 * m + skl <= sql).astype(bf16) for m in range(4)], axis=1
    )  # [128, 4, 512]

    xTs = [np.ascontiguousarray(x[b].T).astype(bf16) for b in range(B)]
    in_maps = []
    for c in range(N_CORES):
        b, g = divmod(c, CPB)
        hsl = slice(512 * g, 512 * (g + 1))
        in_maps.append(
            {
                "xT": xTs[b],
                "wqT": np.ascontiguousarray(wq[hsl, :].T).astype(bf16),
                "wkT": np.ascontiguousarray(wk[hsl, :].T).astype(bf16),
                "wvT": np.ascontiguousarray(wv[hsl, :].T).astype(bf16),
                "woT": np.ascontiguousarray(wo[:, hsl].T).astype(bf16),
                "cosq": cosq,
                "sinq": sinq,
                "mask01": mask01,
                "onesd": np.ones((128, 128), dtype=np.float32),
            }
        )
    return in_maps


def run(inputs, trace=False, **spmd_kwargs):
    """Run the SPMD kernel on 8 cores.  Returns (y_full, BassKernelResults)."""
    nc = build_program()
    in_maps = make_core_inputs(
        inputs["x"], inputs["freqs_cos"], inputs["freqs_sin"],
        inputs["wq"], inputs["wk"], inputs["wv"], inputs["wo"],
    )
    res = bass_utils.run_bass_kernel_spmd(
        nc, in_maps, list(range(N_CORES)), trace=trace, **spmd_kwargs
    )
    out = np.zeros((B, S, D), dtype=np.float32)
    for c in range(N_CORES):
        out[c // CPB] += np.asarray(res.results[c]["y"]).astype(np.float32)
    return out, res


def kernel(**inputs):
    out, _ = run(inputs, trace=False)
    return out


def simulate_core(core_idx, inputs):
    """CoreSim-validate a single core's program; returns its partial y."""
    from concourse.bass_interp import CoreSim

    nc = build_program()
    in_maps = make_core_inputs(
        inputs["x"], inputs["freqs_cos"], inputs["freqs_sin"],
        inputs["wq"], inputs["wk"], inputs["wv"], inputs["wo"],
    )
    sim = CoreSim(nc)
    for name, arr in in_maps[core_idx].items():
        sim.tensor(name)[:] = arr
    sim.simulate()
    return np.array(sim.tensor("y"))
